# revision 20
# baseline (speedup 1.0000x reference)
"""MLA attention Trainium2 kernel: nn_MultiHeadLatentAttention_31722628448847.

Full computation (B=1, T=2048, C=2048, H=16, G=4, Dl=32):
  q  = x @ Wq.T   -> [T, H, G, Dl]
  lk = x @ Wlk.T  -> [T, H, Dl]
  lv = x @ Wlv.T  -> [T, H, Dl]
  scores[h,g,t,s] = (q[t,h,g,:] . lk[s,h,:]) / sqrt(128)
  probs = softmax_s(scores)
  attn[t, h,g,:] = sum_s probs * lv[s,h,:]
  out = attn @ Wo.T
Sharding: 2 heads per core (8 cores); each core computes a full-width
partial of the output projection; partials are summed on the host.

v3 design (t-chunk TC=256, 8 chunks):
 - PSUM: sc pool 3 x [128,1024] f32 (6 banks) + av pool 2 x [128,512]
   (2 banks; each av tile packs two groups' accumulators side by side).
 - per (sb, h): ONE sc tile holds all 4 groups' scores (4 row-tiled MMs,
   N=256 each, issued back-to-back -> concurrent); ONE exp op N=1024.
 - exp: ScalarE Exp for 3/4 of units; DVE Schraudolph (one tensor_scalar
   fp32->int16 with bf16 bitcast, max rel err ~3.3%) for sb%4==3 units.
 - AV: per (sb, h, gp): one MM [K=128s, M=33, N=512] covering 2 groups;
   ones column in lv gives softmax denominators.
 - norm: DVE extracts denominator rows to partition 0, one
   reciprocal_approx_fast per head, bf16 K=1 broadcast matmuls, DVE mult.
 - Wo partials + q projections + bc broadcasts borrow the 3-deep sc
   rotation (dwell < 2 exp units -> no ScalarE stalls).
"""

import numpy as np

T = 2048
C = 2048
HEADS_PER_CORE = 2
DH = 128
DL = 32
G = 4
N_CORES = 8
TC = 256  # t-chunk
PC = 512  # projection chunk
SCALE = 1.0 / np.sqrt(np.float32(DH))
EXPA = 184.6635  # bf16 Schraudolph: bits = round(x*EXPA*SCALE + EXPB)
EXPB = 16250.5


def build_program(t=T, c=C):
    import concourse.mybir as mybir
    import concourse.tile as tile
    from concourse import bacc
    from concourse.masks import make_identity

    bf16 = mybir.dt.bfloat16
    f32 = mybir.dt.float32

    nc = bacc.Bacc("TRN2", target_bir_lowering=False, debug=False, num_devices=1)

    n_cb = c // 128
    QCOLS = HEADS_PER_CORE * DH  # 256
    KCOLS = HEADS_PER_CORE * DL  # 64

    xT_d = nc.dram_tensor("xT", [c, t], bf16, kind="ExternalInput").ap()
    wqT_d = nc.dram_tensor("wqT", [c, QCOLS], bf16, kind="ExternalInput").ap()
    wkT_d = nc.dram_tensor("wkT", [c, QCOLS], bf16, kind="ExternalInput").ap()
    wvT_d = nc.dram_tensor("wvT", [c, KCOLS], bf16, kind="ExternalInput").ap()
    woT_d = nc.dram_tensor("woT", [QCOLS, c], bf16, kind="ExternalInput").ap()
    out_d = nc.dram_tensor("out", [t, c], bf16, kind="ExternalOutput").ap()

    with tile.TileContext(nc) as tc_:
        _emit(
            nc, tc_, tile, mybir, make_identity, bf16, f32,
            xT_d, wqT_d, wkT_d, wvT_d, woT_d, out_d,
            t, c, n_cb, QCOLS, KCOLS,
        )
    nc.compile()
    return nc


def _emit(
    nc, tc_, tile, mybir, make_identity, bf16, f32,
    xT_d, wqT_d, wkT_d, wvT_d, woT_d, out_d,
    t, c, n_cb, QCOLS, KCOLS,
):
    H = HEADS_PER_CORE
    n_sb = t // 128  # 16
    n_tc = t // TC  # 8
    EXP = mybir.ActivationFunctionType.Exp
    i16 = mybir.dt.int16
    MULT = mybir.AluOpType.mult
    ADD = mybir.AluOpType.add
    from contextlib import ExitStack

    ctx = ExitStack()
    with ctx:
        # ---------------- persistent SBUF inputs ----------------
        wpool = ctx.enter_context(tc_.tile_pool(name="wpool", bufs=1))
        xT_sb, wqT_sb, wkT_sb, wvT_sb = [], [], [], []
        for kb in range(n_cb):
            xt = wpool.tile([128, t], bf16, name=f"xT{kb}")
            dma_eng = nc.sync if kb % 2 == 0 else nc.scalar
            dma_eng.dma_start(xt[:], xT_d[kb * 128 : (kb + 1) * 128, :])
            xT_sb.append(xt)
            wk = wpool.tile([128, QCOLS], bf16, name=f"wkT{kb}")
            nc.sync.dma_start(wk[:], wkT_d[kb * 128 : (kb + 1) * 128, :])
            wkT_sb.append(wk)
            wq = wpool.tile([128, QCOLS], bf16, name=f"wqT{kb}")
            nc.sync.dma_start(wq[:], wqT_d[kb * 128 : (kb + 1) * 128, :])
            wqT_sb.append(wq)
            wv = wpool.tile([128, KCOLS], bf16, name=f"wvT{kb}")
            nc.sync.dma_start(wv[:], wvT_d[kb * 128 : (kb + 1) * 128, :])
            wvT_sb.append(wv)
        woT_sb = []
        for h in range(H):
            wo = wpool.tile([128, c], bf16, name=f"woT{h}")
            nc.sync.dma_start(wo[:], woT_d[h * 128 : (h + 1) * 128, :])
            woT_sb.append(wo)

        ident = wpool.tile([128, 128], bf16, name="ident")
        make_identity(nc, ident[:])
        ones1 = wpool.tile([1, DL], bf16, name="ones1")
        nc.vector.memset(ones1[:], 1.0)

        # ---------------- SBUF working tiles ----------------
        apool = ctx.enter_context(tc_.tile_pool(name="apool", bufs=1))
        # qz: block-diagonal zero-padded q layout. Per 512-col group (one
        # t-chunk, one gp): cols 0:256 hold q of the even group on its own
        # 32-row band, cols 256:512 the odd group; all other rows zero.
        qz = [apool.tile([128, 2 * t], bf16, name=f"qz{h}") for h in range(H)]
        for h in range(H):
            nc.vector.memset(qz[h][:], 0.0)
        lkT = [apool.tile([128, t], bf16, name=f"lkT{h}") for h in range(H)]
        lv_all = apool.tile([128, 66 * n_sb], bf16, name="lv_all")
        lv_sb = [lv_all[:, 66 * sb : 66 * (sb + 1)] for sb in range(n_sb)]
        lvT_tmp = apool.tile([KCOLS, t], bf16, name="lvT_tmp")

        # persistent denominator staging (serialized across chunks by WAR);
        # one [1, 512] tile per (h, gp) -- custom-DVE ops need offset-0 APs
        den = [
            [apool.tile([1, 2 * TC], f32, name=f"den{h}_{gp}") for gp in range(2)]
            for h in range(H)
        ]
        rec = [
            [apool.tile([1, 2 * TC], f32, name=f"rec{h}_{gp}") for gp in range(2)]
            for h in range(H)
        ]
        recb = [
            [apool.tile([1, 2 * TC], bf16, name=f"recb{h}_{gp}") for gp in range(2)]
            for h in range(H)
        ]

        expool = ctx.enter_context(tc_.tile_pool(name="expool", bufs=12))
        atpool = ctx.enter_context(tc_.tile_pool(name="atpool", bufs=4))
        bcspool = ctx.enter_context(tc_.tile_pool(name="bcspool", bufs=3))
        otpool = ctx.enter_context(tc_.tile_pool(name="otpool", bufs=6))

        scpool = ctx.enter_context(tc_.tile_pool(name="scpool", bufs=3, space="PSUM"))
        avpool = ctx.enter_context(tc_.tile_pool(name="avpool", bufs=2, space="PSUM"))

        # ---------------- emission helpers ----------------
        def proj_chunk(dst_sb, w_sb, col0, ncols, c0, cw, name, qz_mode=False):
            """dst_sb[:, c0:c0+cw] = (W.T @ x) chunk, bf16 (cw <= 512).

            qz_mode: scatter the [32,TC] g-bands into the block-diagonal
            qz slot layout (tci = c0 // TC)."""
            ps = scpool.tile([128, 2 * PC], f32, name=f"ps_{name}", tag="sc")
            for kb in range(n_cb):
                nc.tensor.matmul(
                    ps[0:ncols, 0:cw],
                    w_sb[kb][:, col0 : col0 + ncols],
                    xT_sb[kb][:, c0 : c0 + cw],
                    start=(kb == 0),
                    stop=(kb == n_cb - 1),
                )
            if qz_mode:
                nslot = cw // TC
                for g in range(G):
                    dst = dst_sb[g * DL : (g + 1) * DL, :].rearrange(
                        "p (s gi c) -> p s gi c", gi=2, c=TC
                    )[:, c0 // TC : c0 // TC + nslot, g % 2 : g % 2 + 1, :]
                    srcv = ps[g * DL : (g + 1) * DL, 0:cw].rearrange(
                        "p (s c) -> p s c", c=TC
                    )
                    nc.vector.tensor_copy(dst, srcv)
            else:
                nc.vector.tensor_copy(
                    dst_sb[0:ncols, c0 : c0 + cw], ps[0:ncols, 0:cw]
                )

        def lv_chunk(nch):
            """lv projection chunk (PC cols) + transposes into lv_sb."""
            ps = scpool.tile([128, 2 * PC], f32, name="ps_lv", tag="sc")
            for kb in range(n_cb):
                nc.tensor.matmul(
                    ps[0:KCOLS, 0:PC],
                    wvT_sb[kb][:],
                    xT_sb[kb][:, nch * PC : (nch + 1) * PC],
                    start=(kb == 0),
                    stop=(kb == n_cb - 1),
                )
            nc.vector.tensor_copy(
                lvT_tmp[:, nch * PC : (nch + 1) * PC], ps[0:KCOLS, 0:PC]
            )
            for j in range(PC // 128):
                sb = nch * (PC // 128) + j
                pt = scpool.tile([128, 2 * PC], bf16, name="ps_lvT", tag="sc")
                nc.tensor.transpose(
                    pt[:, 0:KCOLS],
                    lvT_tmp[:, sb * 128 : (sb + 1) * 128],
                    ident[0:KCOLS, 0:KCOLS],
                )
                nc.vector.tensor_copy(lv_sb[sb][:, 0:DL], pt[:, 0:DL])
                nc.vector.tensor_copy(
                    lv_sb[sb][:, DL + 1 : 2 * DL + 1], pt[:, DL : 2 * DL]
                )
                nc.vector.memset(lv_sb[sb][:, DL : DL + 1], 1.0)
                nc.vector.memset(lv_sb[sb][:, 2 * DL + 1 : 2 * DL + 2], 1.0)

        def scores_exp(tci, sb, h):
            """2 block-diag K=64 score MMs (one PSUM bank each) + exp."""
            sc = scpool.tile([128, 4 * TC], f32, name="sc", tag="sc")
            for gp in range(2):
                nc.tensor.matmul(
                    sc[:, gp * 2 * TC : (gp + 1) * 2 * TC],
                    lkT[h][gp * 64 : (gp + 1) * 64, sb * 128 : (sb + 1) * 128],
                    qz[h][gp * 64 : (gp + 1) * 64, tci * 2 * TC : (tci + 1) * 2 * TC],
                    start=True,
                    stop=True,
                    tile_position=(gp * 64, 0),
                )
            ex = expool.tile([128, 4 * TC], bf16, name="ex", tag="ex")
            if sb % 2 == 0 and h == 1:
                # Schraudolph bf16 exp on DVE (max rel err ~3.3%)
                nc.vector.tensor_scalar(
                    ex[:].bitcast(i16), sc[:], float(EXPA * SCALE), EXPB, MULT, ADD
                )
            else:
                nc.scalar.activation(ex[:], sc[:], EXP, scale=float(SCALE))
            return ex

        def av_accum(sb, av, ex_h):
            """4 AV MMs (h x gp), each covering 2 groups (N=512)."""
            for gp in range(2):
                for h in range(H):
                    nc.tensor.matmul(
                        av[gp][h * 64 : h * 64 + DL + 1, :],
                        lv_sb[sb][:, h * (DL + 1) : (h + 1) * (DL + 1)],
                        ex_h[h][:, gp * 2 * TC : (gp + 1) * 2 * TC],
                        start=(sb == 0),
                        stop=(sb == n_sb - 1),
                        skip_group_check=True,
                        tile_position=(0, h * 64),
                    )

        def norm(av, at):
            """av pair tiles -> normalized bf16 at[h] tiles (one t-chunk)."""
            for h in range(H):
                r = DL + h * 64
                for gp in range(2):
                    nc.vector.tensor_scalar(
                        den[h][gp][:], av[gp][r : r + 1, :], 1.0, None, MULT
                    )
                    nc.vector.reciprocal_approx_fast(rec[h][gp][:], den[h][gp][:])
                    nc.vector.tensor_scalar(
                        recb[h][gp][:], rec[h][gp][:], 1.0, None, MULT
                    )
            bc = scpool.tile([128, 4 * TC], f32, name="bc", tag="sc")
            for gp in range(2):
                for h in range(H):
                    nc.tensor.matmul(
                        bc[h * 64 : h * 64 + DL, gp * 2 * TC : (gp + 1) * 2 * TC],
                        ones1[:],
                        recb[h][gp][:],
                        start=True,
                        stop=True,
                        skip_group_check=True,
                        tile_position=(0, h * 64),
                    )
            bcs = bcspool.tile([128, 4 * TC], f32, name="bcs", tag="bcs")
            for h in range(H):
                nc.vector.tensor_copy(
                    bcs[h * 64 : h * 64 + DL, :], bc[h * 64 : h * 64 + DL, :]
                )
            for gp in range(2):
                for h in range(H):
                    for gi in range(2):
                        g = 2 * gp + gi
                        nc.vector.tensor_tensor(
                            at[h][g * DL : (g + 1) * DL, :],
                            av[gp][h * 64 : h * 64 + DL, gi * TC : (gi + 1) * TC],
                            bcs[h * 64 : h * 64 + DL, gp * 2 * TC + gi * TC : gp * 2 * TC + (gi + 1) * TC],
                            MULT,
                        )

        def wo_chunk(tci, at, ck):
            """output chunk ck (of 4): t-block tb, 1024 out cols oc2."""
            tb, oc2 = divmod(ck, 2)
            t0 = tci * TC + tb * 128
            wos = scpool.tile([128, 4 * TC], f32, name="wos", tag="sc")
            for nh in range(2):
                for h in range(H):
                    nc.tensor.matmul(
                        wos[:, nh * PC : (nh + 1) * PC],
                        at[h][:, tb * 128 : (tb + 1) * 128],
                        woT_sb[h][:, oc2 * 2 * PC + nh * PC : oc2 * 2 * PC + (nh + 1) * PC],
                        start=(h == 0),
                        stop=(h == H - 1),
                    )
            ot = otpool.tile([128, 2 * PC], bf16, name="ot", tag="ot")
            nc.vector.tensor_copy(ot[:], wos[:, 0 : 2 * PC])
            nc.sync.dma_start(
                out_d[t0 : t0 + 128, oc2 * 2 * PC : (oc2 + 1) * 2 * PC], ot[:]
            )

        def intro_phase1():
            """kb-paced: lk chunks 0-1 (both heads) + q chunk 0 (both
            heads), interleaved kb-outer so matmuls pipeline with the xT
            DMA stream. First-exp dependencies (lk c0 + q c0) land right
            at DMA completion."""
            pA = scpool.tile([128, 2 * PC], f32, name="ps_ilkA", tag="sc")
            pC = scpool.tile([128, 2 * PC], f32, name="ps_ilkC", tag="sc")
            pq = [
                avpool.tile([128, 2 * TC], f32, name=f"ps_iq{h}", tag="av")
                for h in range(H)
            ]
            for kb in range(n_cb):
                # first-exp critical regions first
                nc.tensor.matmul(
                    pA[:, 0:PC], wkT_sb[kb][:, 0:128], xT_sb[kb][:, 0:PC],
                    start=(kb == 0), stop=(kb == n_cb - 1),
                )
                nc.tensor.matmul(
                    pC[:, 0:PC], wkT_sb[kb][:, 128:256], xT_sb[kb][:, 0:PC],
                    start=(kb == 0), stop=(kb == n_cb - 1),
                )
                for h in range(H):
                    nc.tensor.matmul(
                        pq[h][:, 0:TC],
                        wqT_sb[kb][:, h * 128 : (h + 1) * 128],
                        xT_sb[kb][:, 0:TC],
                        start=(kb == 0), stop=(kb == n_cb - 1),
                    )
                nc.tensor.matmul(
                    pA[:, PC : 2 * PC], wkT_sb[kb][:, 0:128],
                    xT_sb[kb][:, PC : 2 * PC],
                    start=(kb == 0), stop=(kb == n_cb - 1),
                )
                nc.tensor.matmul(
                    pC[:, PC : 2 * PC], wkT_sb[kb][:, 128:256],
                    xT_sb[kb][:, PC : 2 * PC],
                    start=(kb == 0), stop=(kb == n_cb - 1),
                )
            for nch in range(2):
                nc.vector.tensor_copy(
                    lkT[0][:, nch * PC : (nch + 1) * PC],
                    pA[:, nch * PC : (nch + 1) * PC],
                )
                nc.vector.tensor_copy(
                    lkT[1][:, nch * PC : (nch + 1) * PC],
                    pC[:, nch * PC : (nch + 1) * PC],
                )
            for h in range(H):
                for g in range(G):
                    dst = qz[h][g * DL : (g + 1) * DL, :].rearrange(
                        "p (s gi c) -> p s gi c", gi=2, c=TC
                    )[:, 0:1, g % 2 : g % 2 + 1, :]
                    nc.vector.tensor_copy(
                        dst, pq[h][g * DL : (g + 1) * DL, 0:TC]
                    )

        # ---------------- emission ----------------
        intro_phase1()
        for h in range(H):
            proj_chunk(lkT[h], wkT_sb, h * 128, 128, 2 * PC, PC, f"lk{h}c2")
            proj_chunk(lkT[h], wkT_sb, h * 128, 128, 3 * PC, PC, f"lk{h}c3")

        prev = None
        for tci in range(n_tc):
            at = [
                atpool.tile([128, TC], bf16, name=f"at{h}", tag="at")
                for h in range(H)
            ]
            items = []
            if tci == 0:
                items += [lambda nch=nch: lv_chunk(nch) for nch in range(t // PC)]
            else:
                pav, pat, ptci = prev
                items += [lambda: norm(pav, pat)]
            if tci % 2 == 0 and tci < n_tc - 1:
                # q chunk covering the next two t-chunks (clamped at the end)
                qw = min(2 * TC, t - (tci + 1) * TC)
                items += [
                    lambda h=h, qw=qw: proj_chunk(
                        qz[h], wqT_sb, h * 128, 128, (tci + 1) * TC, qw,
                        f"q{h}", qz_mode=True,
                    )
                    for h in range(H)
                ]
            if tci > 0:
                items += [lambda k=k: wo_chunk(ptci, pat, k) for k in range(4)]
            # delay av alloc so the in-order PE queue never blocks on the
            # previous chunk's norm (DVE) latency
            av_after = 2 if tci == 0 else 3
            av_after = min(av_after, len(items))
            av = None
            backlog = []
            ex_sb = {}
            emitted = 0
            for sb in range(n_sb):
                for h in range(H):
                    ex_sb[(sb, h)] = scores_exp(tci, sb, h)
                if items:
                    items.pop(0)()
                    emitted += 1
                if av is None:
                    backlog.append(sb)
                    if emitted >= av_after:
                        av = [
                            avpool.tile([128, 2 * TC], f32, name=f"av{gp}", tag="av")
                            for gp in range(2)
                        ]
                        for b in backlog:
                            av_accum(b, av, [ex_sb[(b, h)] for h in range(H)])
                else:
                    av_accum(sb, av, [ex_sb[(sb, h)] for h in range(H)])
            while items:
                items.pop(0)()
            prev = (av, at, tci)

        pav, pat, ptci = prev
        norm(pav, pat)
        for k in range(4):
            wo_chunk(ptci, pat, k)


# ---------------- host side ----------------


def shard_inputs(x, Wq, Wlk, Wlv, Wo):
    """Returns per-core input dicts (bf16, pre-transposed)."""
    import ml_dtypes

    bf = ml_dtypes.bfloat16
    X = np.ascontiguousarray(x.reshape(-1, x.shape[-1]))  # [T, C]
    xT = np.ascontiguousarray(X.T).astype(bf)
    maps = []
    for core in range(N_CORES):
        h0 = core * HEADS_PER_CORE
        qr = slice(h0 * DH, (h0 + HEADS_PER_CORE) * DH)
        wk_blocks = []
        for h in range(HEADS_PER_CORE):
            hr = slice((h0 + h) * DL, (h0 + h + 1) * DL)
            wk_blocks.append(np.tile(Wlk[hr, :].T, (1, G)))  # [C, 128]
        kr = slice(h0 * DL, (h0 + HEADS_PER_CORE) * DL)
        maps.append(
            {
                "xT": xT,
                "wqT": np.ascontiguousarray(Wq[qr, :].T).astype(bf),
                "wkT": np.ascontiguousarray(
                    np.concatenate(wk_blocks, axis=1)
                ).astype(bf),
                "wvT": np.ascontiguousarray(Wlv[kr, :].T).astype(bf),
                "woT": np.ascontiguousarray(Wo[:, qr].T).astype(bf),
            }
        )
    return maps


_CACHE = {}


def kernel(x, Wq, Wk, Wv, Wlk, Wlv, Wo):
    """Full-input entry point. Wk/Wv are unused by the reference forward."""
    if "nc" not in _CACHE:
        _CACHE["nc"] = build_program()
    nc = _CACHE["nc"]
    from concourse.bass_utils import run_bass_kernel_spmd

    in_maps = shard_inputs(
        np.asarray(x, dtype=np.float32),
        np.asarray(Wq, dtype=np.float32),
        np.asarray(Wlk, dtype=np.float32),
        np.asarray(Wlv, dtype=np.float32),
        np.asarray(Wo, dtype=np.float32),
    )
    res = run_bass_kernel_spmd(nc, in_maps, list(range(N_CORES)))
    out = np.zeros((T, C), dtype=np.float32)
    for r in res.results:
        out += r["out"].astype(np.float32)
    return out.reshape(1, T, C)


def _cache_get():
    return _CACHE["nc"]
